# revision 74
# baseline (speedup 1.0000x reference)
"""GraphTransformerLayer (PyG TransformerConv style) on 8 trn2 NeuronCores.

Nodes are sharded 1/8 per core; each core owns every edge whose destination
falls in its node range, grouped per 128-node destination block, so the
segment-softmax + scatter-add need no cross-core reduction. K/V projections
are computed on-device from each core's own node slice and AllGathered
across cores, so the full node-feature matrix never crosses the (slow)
host->device link. Edge attributes travel as fp8-e4m3 and are upcast +
transposed on-device. Per 128-node block, segment-softmax + scatter-add are
one-hot matmuls accumulating into PSUM; LayerNorm/FFN are node-parallel.
"""
import numpy as np
import ml_dtypes

P = 128
H = 8
C = 16
GROUP = 4
N_CORES = 8

_RUNTIME_CACHE = {}


def _prep_x(x):
    x = np.ascontiguousarray(np.asarray(x, dtype=np.float32))
    N, D = x.shape
    Nc = N // N_CORES
    NB = (Nc + P - 1) // P
    Npad = NB * P
    meta = dict(N=N, D=D, Nc=Nc, NB=NB, Npad=Npad)

    def x_chunk(c):  # per-core bf16 conversion so it pipelines with the wire
        xc = np.zeros((Npad, D), ml_dtypes.bfloat16)
        xc[:Nc] = x[c * Nc:(c + 1) * Nc].astype(ml_dtypes.bfloat16)
        return xc

    return meta, x_chunk


def _prep_edges(edge_index, edge_attr, meta):
    N, Nc, NB, Npad = meta["N"], meta["Nc"], meta["NB"], meta["Npad"]
    E = edge_index.shape[1]
    ED = edge_attr.shape[1]

    src = np.asarray(edge_index[0]).astype(np.int64, copy=False)
    dst = np.asarray(edge_index[1]).astype(np.int64, copy=False)

    # Degree-balanced placement: snake each core's nodes (sorted by
    # in-degree) across its 49 blocks so per-block edge counts even out,
    # which lowers K (tiles per block) and with it the attr/eidx padding.
    # The device reads x and writes out through the `perm` table, so no
    # host-side gather/scatter of the 25MB feature/output arrays is needed.
    deg = np.bincount(dst, minlength=N).reshape(N_CORES, Nc)
    order_c = np.argsort(-deg, axis=1, kind="stable")      # [8, Nc]
    i = np.arange(Nc)
    blk_i = i % NB
    blk_i = np.where((i // NB) % 2 == 1, NB - 1 - blk_i, blk_i)
    pos_i = blk_i * P + i // NB                            # padded position
    new_rel = np.empty((N_CORES, Nc), np.int64)
    np.put_along_axis(new_rel, order_c,
                      np.broadcast_to(pos_i, (N_CORES, Nc)), axis=1)
    perm = np.full((N_CORES, Npad), -1, np.int64)
    np.put_along_axis(perm, new_rel,
                      np.broadcast_to(np.arange(Nc), (N_CORES, Nc)), axis=1)
    pad_rows = np.arange(Nc, Npad)
    for c in range(N_CORES):
        perm[c, perm[c] == -1] = pad_rows
    perm_g = perm.reshape(N_CORES * Npad, 1).astype(np.int32)

    core_e = dst // Nc
    rel_new = new_rel[core_e, dst - core_e * Nc]
    segid_e = core_e * NB + rel_new // P   # (core, block) id after placement
    order = np.argsort(segid_e)
    seg_s = segid_e[order]
    src_s = src[order]
    rel_s = rel_new[order]
    cnt = np.bincount(seg_s, minlength=N_CORES * NB)
    K = max(1, -(-int(cnt.max()) // P))   # 128-edge tiles per block
    Ecp = NB * K * P
    seg_start = np.concatenate(([0], np.cumsum(cnt[:-1])))
    slot = seg_s * (K * P) + (np.arange(E, dtype=np.int64) - seg_start[seg_s])

    # one packed int per edge: src_row * 256 + (dstrel + 1); pad slots are 0,
    # so the one-hot (iota base 1) never fires and the kv gather hits row 0.
    # Values < 2^24, shipped as 3 little-endian bytes per edge.
    src_core = src_s // Nc
    src_pos = new_rel[src_core, src_s - src_core * Nc]
    vals = np.zeros(N_CORES * Ecp, np.int32)
    vals[slot] = ((src_core * Npad + src_pos) * 256
                  + rel_s % P + 1).astype(np.int32)
    eidx = np.empty((N_CORES * Ecp, 3), np.uint8)
    eidx[:, 0] = vals & 255
    eidx[:, 1] = (vals >> 8) & 255
    eidx[:, 2] = vals >> 16

    attr_np = np.asarray(edge_attr, np.float32)
    core_bounds = np.searchsorted(seg_s, np.arange(N_CORES + 1) * NB)

    def attr_chunk(c):  # heavy fp8 conversion, per core so it pipelines
        lo, hi = core_bounds[c], core_bounds[c + 1]
        chunk = np.zeros((Ecp, ED), ml_dtypes.float8_e4m3)
        chunk[slot[lo:hi] - c * Ecp] = attr_np[order[lo:hi]].astype(
            ml_dtypes.float8_e4m3)
        return chunk

    meta.update(ED=ED, K=K, Ecp=Ecp)
    return attr_chunk, eidx, perm_g


def _build(meta):
    import concourse.bacc as bacc
    import concourse.bass as bass
    import concourse.tile as tile
    from concourse import mybir
    from concourse.masks import make_identity

    f32 = mybir.dt.float32
    bf16 = mybir.dt.bfloat16
    f8 = mybir.dt.float8e4
    i32 = mybir.dt.int32
    i8 = mybir.dt.int8
    u8 = mybir.dt.uint8
    D, ED = meta["D"], meta["ED"]
    NB, Npad, K, Ecp = meta["NB"], meta["Npad"], meta["K"], meta["Ecp"]

    nc = bacc.Bacc("TRN2", target_bir_lowering=False, debug=False,
                   num_devices=N_CORES)

    x_own = nc.dram_tensor("x_own", [Npad, D], bf16, kind="ExternalInput").ap()
    attr = nc.dram_tensor("attr", [Ecp, ED], f8, kind="ExternalInput").ap()
    eidx = nc.dram_tensor("eidx", [Ecp, 3], u8, kind="ExternalInput").ap()
    permt = nc.dram_tensor("perm", [Npad, 1], i32, kind="ExternalInput").ap()
    Wqkv = nc.dram_tensor("Wqkv", [D, 3 * D], bf16, kind="ExternalInput").ap()
    We = nc.dram_tensor("We", [ED, D], bf16, kind="ExternalInput").ap()
    Wskip = nc.dram_tensor("Wskip", [D, D], bf16, kind="ExternalInput").ap()
    Wf1 = nc.dram_tensor("Wf1", [D, 4 * D], bf16, kind="ExternalInput").ap()
    Wf2 = nc.dram_tensor("Wf2", [4 * D, D], bf16, kind="ExternalInput").ap()
    bf1 = nc.dram_tensor("bf1", [4, D], f32, kind="ExternalInput").ap()
    bqkv = nc.dram_tensor("bqkv", [1, 3 * D], f32, kind="ExternalInput").ap()
    # rows: bskip, bf2, g1, b1, g2, b2, (2 spare)
    bvec = nc.dram_tensor("bvec", [8, D], f32, kind="ExternalInput").ap()
    out = nc.dram_tensor("out", [Npad, D], i8, kind="ExternalOutput").ap()

    def ap_append(ap, n):
        a = ap.copy()
        a.ap = a.ap + [[0, n]]
        return a

    def ins_mid(ap, pos, n):
        a = ap.copy()
        a.ap = a.ap[:pos] + [[0, n]] + a.ap[pos:]
        return a

    def bcast_row(dram_ap, row, width):
        """AP replicating one [width] DRAM row across P partitions."""
        return bass.AP(tensor=dram_ap.tensor, offset=dram_ap.offset + row * width,
                       ap=[[0, P], [1, width]])

    from contextlib import ExitStack
    _ctx = ExitStack()
    with tile.TileContext(nc) as tc:
        const = _ctx.enter_context(tc.tile_pool(name="const", bufs=1))
        sb = _ctx.enter_context(tc.tile_pool(name="sb", bufs=3))
        sb2 = _ctx.enter_context(tc.tile_pool(name="sb2", bufs=2))
        ps_pool = _ctx.enter_context(tc.tile_pool(name="ps", bufs=2, space="PSUM"))
        tp_ps = _ctx.enter_context(tc.tile_pool(name="tpps", bufs=1, space="PSUM"))
        ep_ps = _ctx.enter_context(tc.tile_pool(name="epps", bufs=1, space="PSUM"))
        acc_pool = _ctx.enter_context(tc.tile_pool(name="acc", bufs=2, space="PSUM"))
        dram = _ctx.enter_context(tc.tile_pool(name="dram", bufs=1, space="DRAM"))

        W3_sb = const.tile([D, 3 * D], bf16)
        nc.sync.dma_start(out=W3_sb[:], in_=Wqkv[:, :])
        We_sb = const.tile([ED, D], bf16)
        nc.sync.dma_start(out=We_sb[:], in_=We[:, :])
        Wskip_sb = const.tile([D, D], bf16)
        nc.sync.dma_start(out=Wskip_sb[:], in_=Wskip[:, :])
        Wf1_sb = const.tile([D, 4 * D], bf16)
        nc.sync.dma_start(out=Wf1_sb[:], in_=Wf1[:, :])
        Wf2_sb = const.tile([D, 4, D], bf16)
        for j in range(4):
            nc.sync.dma_start(out=Wf2_sb[:, j, :], in_=Wf2[j * D:(j + 1) * D, :])
        bf1_sb = const.tile([D, 4], f32)
        for j in range(4):
            nc.sync.dma_start(out=bf1_sb[:, j:j + 1], in_=bf1[j, :, None])
        b3_bc = const.tile([P, 3 * D], f32)
        nc.sync.dma_start(out=b3_bc[:], in_=bcast_row(bqkv, 0, 3 * D))
        brows = const.tile([P, 6, D], f32)
        for r in range(6):
            nc.sync.dma_start(out=brows[:, r, :], in_=bcast_row(bvec, r, D))
        bskip_bc = brows[:, 0, :]
        bf2_bc = brows[:, 1, :]
        g1_bc = brows[:, 2, :]
        b1_bc = brows[:, 3, :]
        g2_bc = brows[:, 4, :]
        b2_bc = brows[:, 5, :]
        ident = const.tile([P, P], f32)
        make_identity(nc, ident[:])
        ident_bf = const.tile([P, P], bf16)
        nc.vector.tensor_copy(out=ident_bf[:], in_=ident[:])
        iota_t = const.tile([P, P], i32)
        nc.gpsimd.iota(iota_t[:], pattern=[[1, P]], base=1, channel_multiplier=0)
        eps_t = const.tile([P, 1], f32)
        nc.vector.memset(eps_t[:], 1e-5)
        perm_sb = const.tile([P, NB], i32)   # perm_sb[p, t] = perm[t*P + p]
        nc.sync.dma_start(
            out=perm_sb[:, :],
            in_=bass.AP(tensor=permt.tensor, offset=permt.offset,
                        ap=[[1, P], [P, NB]]))

        kv_own = dram.tile([Npad, 2 * D], f32)
        kv_all = dram.tile([N_CORES * Npad, 2 * D], f32)
        q_t = dram.tile([Npad, D], f32)

        # ---- phase A: per-block K/V/Q projections of own node slice ----
        for t in range(NB):
            xo = sb.tile([P, D], bf16, tag="xo")
            nc.gpsimd.indirect_dma_start(
                out=xo[:], out_offset=None, in_=x_own[:, :],
                in_offset=bass.IndirectOffsetOnAxis(ap=perm_sb[:, t:t + 1], axis=0))
            tp = tp_ps.tile([P, D], bf16, tag="tpe")
            nc.tensor.transpose(tp[:], xo[:], ident_bf[:])
            xt = sb.tile([P, D], bf16, tag="xt")
            nc.vector.tensor_copy(out=xt[:], in_=tp[:])
            p3 = ps_pool.tile([P, 3 * D], f32, tag="eps")
            nc.tensor.matmul(p3[:], lhsT=xt[:], rhs=W3_sb[:], start=True, stop=True)
            kvq = sb.tile([P, 3 * D], f32, tag="kvo")
            nc.vector.tensor_tensor(out=kvq[:], in0=p3[:], in1=b3_bc[:],
                                    op=mybir.AluOpType.add)
            nc.sync.dma_start(out=kv_own[t * P:(t + 1) * P, :], in_=kvq[:, 0:2 * D])
            nc.sync.dma_start(out=q_t[t * P:(t + 1) * P, :], in_=kvq[:, 2 * D:3 * D])

        # ---- all-gather the kv table across cores ----
        nc.gpsimd.collective_compute(
            "AllGather", mybir.AluOpType.bypass,
            replica_groups=[list(range(N_CORES))],
            ins=[kv_own.opt()], outs=[kv_all.opt()])
        tc.strict_bb_all_engine_barrier()

        # ---- phase C: edge aggregation + node epilogue per 128-node block ----
        n_full, rem = divmod(K, GROUP)
        groups = [GROUP] * n_full + ([rem] if rem else [])
        for b in range(NB):
            acc = acc_pool.tile([P, 136], f32, tag="acc")
            kk = 0
            for gi, G in enumerate(groups):
                e0 = (b * K + kk) * P
                eb8 = sb.tile([P, G, 3], u8, tag="eb8")
                src_dram = eidx[e0:e0 + G * P, :]
                nc.sync.dma_start(
                    out=eb8[:, :, :],
                    in_=bass.AP(tensor=src_dram.tensor, offset=src_dram.offset,
                                ap=[[3, P], [P * 3, G], [1, 3]]))
                ebt = sb.tile([P, G, 3], i32, tag="ebt")
                nc.vector.tensor_copy(out=ebt[:], in_=eb8[:])
                pk = sb.tile([P, G], i32, tag="idx")
                nc.vector.tensor_scalar(out=pk[:], in0=ebt[:, :, 2],
                                        scalar1=256, scalar2=None,
                                        op0=mybir.AluOpType.mult)
                nc.vector.tensor_tensor(out=pk[:], in0=pk[:], in1=ebt[:, :, 1],
                                        op=mybir.AluOpType.add)
                nc.vector.tensor_scalar(out=pk[:], in0=pk[:],
                                        scalar1=256, scalar2=None,
                                        op0=mybir.AluOpType.mult)
                nc.vector.tensor_tensor(out=pk[:], in0=pk[:], in1=ebt[:, :, 0],
                                        op=mybir.AluOpType.add)
                sv = sb.tile([P, G], i32, tag="sv")     # kv row = packed >> 8
                nc.vector.tensor_scalar(out=sv[:], in0=pk[:], scalar1=8,
                                        scalar2=None,
                                        op0=mybir.AluOpType.logical_shift_right)
                dr1 = sb.tile([P, G], i32, tag="dr1")   # dstrel + 1 (0 = pad)
                nc.vector.tensor_scalar(out=dr1[:], in0=pk[:], scalar1=255,
                                        scalar2=None,
                                        op0=mybir.AluOpType.bitwise_and)
                qidx = sb.tile([P, G], i32, tag="qidx")  # q row in own block
                nc.vector.tensor_scalar(out=qidx[:], in0=dr1[:],
                                        scalar1=1, scalar2=b * P - 1,
                                        op0=mybir.AluOpType.max,
                                        op1=mybir.AluOpType.add)
                kv_g = sb.tile([P, G, 2 * D], f32, tag="kvg")
                q_g = sb.tile([P, G, D], f32, tag="qg")
                for g in range(G):
                    nc.gpsimd.indirect_dma_start(
                        out=kv_g[:, g, :], out_offset=None, in_=kv_all[:, :],
                        in_offset=bass.IndirectOffsetOnAxis(ap=sv[:, g:g + 1], axis=0))
                    nc.gpsimd.indirect_dma_start(
                        out=q_g[:, g, :], out_offset=None, in_=q_t[:, :],
                        in_offset=bass.IndirectOffsetOnAxis(ap=qidx[:, g:g + 1], axis=0))
                at = sb.tile([P, G, ED], f8, tag="at")
                attr_dram = attr[e0:e0 + G * P, :]
                nc.sync.dma_start(
                    out=at[:, :, :],
                    in_=bass.AP(tensor=attr_dram.tensor, offset=attr_dram.offset,
                                ap=[[ED, P], [P * ED, G], [1, ED]]))
                at16 = sb.tile([P, G, ED], bf16, tag="at16")
                nc.vector.tensor_copy(out=at16[:], in_=at[:])
                atT = sb.tile([ED, G * P], bf16, tag="atT")
                for g in range(G):
                    tpe = tp_ps.tile([ED, P], bf16, tag="tpe")
                    nc.tensor.transpose(tpe[:], at16[:, g, :], ident_bf[:])
                    nc.vector.tensor_copy(out=atT[:, g * P:(g + 1) * P], in_=tpe[:])
                e_ps = ps_pool.tile([P, G * D], f32, tag="eps")
                for g in range(G):
                    nc.tensor.matmul(e_ps[:, g * D:(g + 1) * D],
                                     lhsT=atT[:, g * P:(g + 1) * P], rhs=We_sb[:],
                                     start=True, stop=True)
                e3 = e_ps[:].rearrange("p (g f) -> p g f", g=G)
                kj = sb.tile([P, G, D], f32, tag="kj")
                nc.vector.tensor_tensor(out=kj[:], in0=kv_g[:, :, 0:D], in1=e3,
                                        op=mybir.AluOpType.add)
                vj = sb.tile([P, G, D], f32, tag="vj")
                nc.vector.tensor_tensor(out=vj[:], in0=kv_g[:, :, D:2 * D], in1=e3,
                                        op=mybir.AluOpType.add)
                prod = sb.tile([P, G, D], f32, tag="prod")
                nc.vector.tensor_tensor(out=prod[:], in0=kj[:], in1=q_g[:],
                                        op=mybir.AluOpType.mult)
                logit = sb.tile([P, G, H], f32, tag="logit")
                nc.vector.tensor_reduce(
                    out=logit[:].rearrange("p g h -> p (g h)"),
                    in_=prod[:].rearrange("p g (h c) -> p (g h) c", h=H),
                    axis=mybir.AxisListType.X, op=mybir.AluOpType.add)
                rhs_st = sb.tile([P, G, 136], f32, tag="rhs")
                nc.scalar.activation(out=rhs_st[:, :, D:D + H], in_=logit[:],
                                     func=mybir.ActivationFunctionType.Exp,
                                     scale=1.0 / np.sqrt(C))
                s4 = ap_append(rhs_st[:, :, D:D + H], C)  # [P, G, H, C]
                nc.vector.tensor_tensor(
                    out=rhs_st[:, :, 0:D].rearrange("p g (h c) -> p g h c", h=H),
                    in0=vj[:].rearrange("p g (h c) -> p g h c", h=H),
                    in1=s4, op=mybir.AluOpType.mult)
                oh = sb.tile([P, G, P], f32, tag="oh")
                nc.vector.tensor_tensor(
                    out=oh[:], in0=ins_mid(iota_t[:], 1, G),
                    in1=ap_append(dr1[:], P),
                    op=mybir.AluOpType.is_equal)
                for g in range(G):
                    nc.tensor.matmul(acc[:, :], lhsT=oh[:, g, :], rhs=rhs_st[:, g, :],
                                     start=(kk + g == 0), stop=(kk + g == K - 1))
                kk += G

            # node-block epilogue
            dn = sb2.tile([P, H], f32, tag="dn")
            nc.vector.tensor_scalar_max(out=dn[:], in0=acc[:, D:D + H], scalar1=1e-30)
            rec = sb2.tile([P, H], f32, tag="rec")
            nc.vector.reciprocal(out=rec[:], in_=dn[:])
            xo2 = sb2.tile([P, D], bf16, tag="xo2")
            nc.gpsimd.indirect_dma_start(
                out=xo2[:], out_offset=None, in_=x_own[:, :],
                in_offset=bass.IndirectOffsetOnAxis(ap=perm_sb[:, b:b + 1], axis=0))
            tr2 = ep_ps.tile([P, D], bf16, tag="tr")
            nc.tensor.transpose(tr2[:], xo2[:], ident_bf[:])
            xt2 = sb2.tile([P, D], bf16, tag="xt2")
            nc.vector.tensor_copy(out=xt2[:], in_=tr2[:])
            xo2f = sb2.tile([P, D], f32, tag="xo2f")
            nc.vector.tensor_copy(out=xo2f[:], in_=xo2[:])
            sk_ps = ep_ps.tile([P, D], f32, tag="mm")
            nc.tensor.matmul(sk_ps[:], lhsT=xt2[:], rhs=Wskip_sb[:], start=True, stop=True)
            h = sb2.tile([P, D], f32, tag="h")
            # agg = acc/denom ; conv = agg + skip + bskip + x
            nc.vector.tensor_tensor(
                out=h[:].rearrange("p (h c) -> p h c", h=H),
                in0=acc[:, 0:D].rearrange("p (h c) -> p h c", h=H),
                in1=ap_append(rec[:], C), op=mybir.AluOpType.mult)
            nc.vector.tensor_tensor(out=h[:], in0=h[:], in1=sk_ps[:], op=mybir.AluOpType.add)
            nc.vector.tensor_tensor(out=h[:], in0=h[:], in1=bskip_bc, op=mybir.AluOpType.add)
            nc.vector.tensor_tensor(out=h[:], in0=h[:], in1=xo2f[:], op=mybir.AluOpType.add)
            # LN1
            st = sb2.tile([P, 6], f32, tag="st")
            nc.vector.bn_stats(out=st[:], in_=h[:])
            mv = sb2.tile([P, 2], f32, tag="mv")
            nc.vector.bn_aggr(out=mv[:], in_=st[:])
            sd = sb2.tile([P, 2], f32, tag="sd")
            nc.scalar.activation(out=sd[:, 0:1], in_=mv[:, 1:2],
                                 func=mybir.ActivationFunctionType.Sqrt,
                                 bias=eps_t[:])
            nc.vector.reciprocal(out=sd[:, 1:2], in_=sd[:, 0:1])
            nc.vector.tensor_scalar(out=h[:], in0=h[:], scalar1=mv[:, 0:1],
                                    scalar2=sd[:, 1:2],
                                    op0=mybir.AluOpType.subtract,
                                    op1=mybir.AluOpType.mult)
            nc.vector.tensor_tensor(out=h[:], in0=h[:], in1=g1_bc, op=mybir.AluOpType.mult)
            nc.vector.tensor_tensor(out=h[:], in0=h[:], in1=b1_bc, op=mybir.AluOpType.add)
            # FFN: h1T = h^T ; out1T_j = Wf1_j^T h1T -> gelu -> out2 += g_j^T Wf2_j
            tr_ps = ep_ps.tile([P, D], f32, tag="tr")
            nc.tensor.transpose(out=tr_ps[:], in_=h[:], identity=ident[:])
            h1T = sb2.tile([P, D], bf16, tag="h1T")
            nc.vector.tensor_copy(out=h1T[:], in_=tr_ps[:])
            o2_ps = ep_ps.tile([P, D], f32, tag="o2ps")
            for j in range(4):
                m1 = ep_ps.tile([P, D], f32, tag="mm")
                nc.tensor.matmul(m1[:], lhsT=Wf1_sb[:, j * D:(j + 1) * D],
                                 rhs=h1T[:], start=True, stop=True)
                gj = sb2.tile([P, D], bf16, tag="gj")
                nc.scalar.activation(out=gj[:], in_=m1[:],
                                     func=mybir.ActivationFunctionType.Gelu,
                                     bias=bf1_sb[:, j:j + 1])
                nc.tensor.matmul(o2_ps[:], lhsT=gj[:], rhs=Wf2_sb[:, j, :],
                                 start=(j == 0), stop=(j == 3))
            h2 = sb2.tile([P, D], f32, tag="h2")
            nc.vector.tensor_tensor(out=h2[:], in0=h[:], in1=o2_ps[:],
                                    op=mybir.AluOpType.add)
            nc.vector.tensor_tensor(out=h2[:], in0=h2[:], in1=bf2_bc, op=mybir.AluOpType.add)
            # LN2
            nc.vector.bn_stats(out=st[:], in_=h2[:])
            nc.vector.bn_aggr(out=mv[:], in_=st[:])
            nc.scalar.activation(out=sd[:, 0:1], in_=mv[:, 1:2],
                                 func=mybir.ActivationFunctionType.Sqrt,
                                 bias=eps_t[:])
            nc.vector.reciprocal(out=sd[:, 1:2], in_=sd[:, 0:1])
            ot = sb2.tile([P, D], f32, tag="ot")
            nc.vector.tensor_scalar(out=ot[:], in0=h2[:], scalar1=mv[:, 0:1],
                                    scalar2=sd[:, 1:2],
                                    op0=mybir.AluOpType.subtract,
                                    op1=mybir.AluOpType.mult)
            nc.vector.tensor_tensor(out=ot[:], in0=ot[:], in1=g2_bc, op=mybir.AluOpType.mult)
            obf = sb2.tile([P, D], f32, tag="obf")
            nc.vector.tensor_tensor(out=obf[:], in0=ot[:], in1=b2_bc, op=mybir.AluOpType.add)
            # int8 output at scale 16: y = round(16*x) via the f32
            # magic-number trick (adding 1.5*2^23 rounds to integer in the
            # mantissa), so the f32->i8 convert below is exact
            t1 = sb2.tile([P, D], f32, tag="t1")
            nc.vector.tensor_scalar(out=t1[:], in0=obf[:], scalar1=16.0,
                                    scalar2=12582912.0,
                                    op0=mybir.AluOpType.mult,
                                    op1=mybir.AluOpType.add)
            t2 = sb2.tile([P, D], f32, tag="t2")
            nc.vector.tensor_scalar(out=t2[:], in0=t1[:], scalar1=-12582912.0,
                                    scalar2=127.0,
                                    op0=mybir.AluOpType.add,
                                    op1=mybir.AluOpType.min)
            ob = sb2.tile([P, D], i8, tag="ob")
            nc.vector.tensor_scalar(out=ob[:], in0=t2[:], scalar1=-127.0,
                                    scalar2=None, op0=mybir.AluOpType.max)
            nc.gpsimd.indirect_dma_start(
                out=out[:, :],
                out_offset=bass.IndirectOffsetOnAxis(ap=perm_sb[:, b:b + 1], axis=0),
                in_=ob[:], in_offset=None)

        _ctx.close()

    nc.compile()
    return nc


class _Runtime:
    pass


def _get_runtime(meta):
    key = (meta["N"], meta["D"], meta["ED"], meta["K"])
    if key in _RUNTIME_CACHE:
        return _RUNTIME_CACHE[key]

    import jax
    import jax.numpy as jnp
    from jax.sharding import Mesh, PartitionSpec, NamedSharding
    from jax.experimental.shard_map import shard_map
    from concourse import mybir
    from concourse.bass2jax import (_bass_exec_p, install_neuronx_cc_hook,
                                    partition_id_tensor)

    nc = _build(meta)
    install_neuronx_cc_hook()
    partition_name = nc.partition_id_tensor.name if nc.partition_id_tensor else None
    in_names, out_names, out_avals, zero_shapes = [], [], [], []
    for alloc in nc.m.functions[0].allocations:
        if not isinstance(alloc, mybir.MemoryLocationSet):
            continue
        name = alloc.memorylocations[0].name
        if alloc.kind == "ExternalInput":
            if name != partition_name:
                in_names.append(name)
        elif alloc.kind == "ExternalOutput":
            shape = tuple(alloc.tensor_shape)
            dtype = mybir.dt.np(alloc.dtype)
            out_names.append(name)
            out_avals.append(jax.core.ShapedArray(shape, dtype))
            zero_shapes.append((shape, dtype))
    n_params = len(in_names)
    n_outs = len(out_avals)
    all_in_names = list(in_names) + list(out_names)
    if partition_name is not None:
        all_in_names.append(partition_name)

    def _body(*args):
        operands = list(args)
        if partition_name is not None:
            operands.append(partition_id_tensor())
        outs = _bass_exec_p.bind(
            *operands,
            out_avals=tuple(out_avals), in_names=tuple(all_in_names),
            out_names=tuple(out_names), lowering_input_output_aliases=(),
            sim_require_finite=True, sim_require_nnan=True, nc=nc)
        return tuple(outs)

    devices = jax.devices()[:N_CORES]
    mesh = Mesh(np.asarray(devices), ("core",))
    sh = NamedSharding(mesh, PartitionSpec("core"))
    in_specs = (PartitionSpec("core"),) * (n_params + n_outs)
    out_specs = (PartitionSpec("core"),) * n_outs
    donate = tuple(range(n_params, n_params + n_outs))
    sharded = jax.jit(
        shard_map(_body, mesh=mesh, in_specs=in_specs, out_specs=out_specs,
                  check_rep=False),
        donate_argnums=donate, keep_unused=True)

    @jax.jit
    def make_zeros():
        return tuple(jax.lax.with_sharding_constraint(
            jnp.zeros((N_CORES * s[0], *s[1:]), d), sh) for s, d in zero_shapes)

    rt = _Runtime()
    rt.nc = nc
    rt.in_names = in_names
    rt.out_names = out_names
    rt.sh = sh
    rt.devices = devices
    rt.sharded = sharded
    rt.make_zeros = make_zeros
    rt.device_put = jax.device_put
    rt.from_shards = jax.make_array_from_single_device_arrays
    _RUNTIME_CACHE[key] = rt
    return rt


def _spot_check(inputs, out, nodes=(0, 25017, 49999)):
    """Recompute a few output rows on the host and compare loosely.

    The device path's quantization error is < ~0.06 per element; transient
    device corruption (seen ~1/18 runs) is off by O(1). A 0.5 threshold
    separates the two with huge margin."""
    import math
    f32 = np.float32
    x = np.asarray(inputs["x"], f32)
    src = np.asarray(inputs["edge_index"][0]).astype(np.int64, copy=False)
    dst = np.asarray(inputs["edge_index"][1]).astype(np.int64, copy=False)
    attr = np.asarray(inputs["edge_attr"], f32)
    W = {k: np.asarray(inputs[k], f32) for k in
         ("Wq", "bq", "Wk", "bk", "Wv", "bv", "We", "Wskip", "bskip",
          "g1", "b1", "g2", "b2", "Wf1", "bf1", "Wf2", "bf2")}
    D = x.shape[1]
    Hn, Cn = H, D // H
    erf = np.vectorize(math.erf)
    for n in nodes:
        e_ids = np.nonzero(dst == n)[0]
        q = (x[n] @ W["Wq"] + W["bq"]).reshape(Hn, Cn)
        if e_ids.size:
            s = src[e_ids]
            e = (attr[e_ids] @ W["We"]).reshape(-1, Hn, Cn)
            kj = (x[s] @ W["Wk"] + W["bk"]).reshape(-1, Hn, Cn) + e
            vj = (x[s] @ W["Wv"] + W["bv"]).reshape(-1, Hn, Cn) + e
            lg = (q[None] * kj).sum(-1) / np.sqrt(Cn)
            lg = lg - lg.max(0, keepdims=True)
            al = np.exp(lg)
            al = al / al.sum(0, keepdims=True)
            agg = (al[..., None] * vj).sum(0).reshape(D)
        else:
            agg = np.zeros(D, f32)
        conv = agg + x[n] @ W["Wskip"] + W["bskip"] + x[n]
        h = (conv - conv.mean()) / np.sqrt(conv.var() + 1e-5)
        h = h * W["g1"] + W["b1"]
        z = h @ W["Wf1"] + W["bf1"]
        f = (0.5 * z * (1.0 + erf(z / np.sqrt(2.0)))) @ W["Wf2"] + W["bf2"]
        y = f + h
        ref = (y - y.mean()) / np.sqrt(y.var() + 1e-5) * W["g2"] + W["b2"]
        if np.abs(out[n] - ref).max() > 0.5:
            return False
    return True


def _core_sharding():
    if "sh" not in _RUNTIME_CACHE:
        import jax
        from jax.sharding import Mesh, PartitionSpec, NamedSharding
        devices = jax.devices()[:N_CORES]
        mesh = Mesh(np.asarray(devices), ("core",))
        _RUNTIME_CACHE["sh"] = (NamedSharding(mesh, PartitionSpec("core")),
                                devices, jax.device_put, jax.device_get)
    return _RUNTIME_CACHE["sh"]


def kernel(**inputs):
    f32 = np.float32
    D = int(np.asarray(inputs["x"]).shape[1])
    sh, devices, device_put, device_get = _core_sharding()

    def rep(a):  # replicate a per-core array along the shard axis
        a = np.asarray(a, f32)
        return np.tile(a, (N_CORES,) + (1,) * (a.ndim - 1))

    def rep_bf(a):
        a = np.asarray(a, f32).astype(ml_dtypes.bfloat16)
        return np.tile(a, (N_CORES,) + (1,) * (a.ndim - 1))

    Wqkv = np.concatenate([np.asarray(inputs["Wk"], f32),
                           np.asarray(inputs["Wv"], f32),
                           np.asarray(inputs["Wq"], f32)], axis=1)
    bqkv = np.concatenate([np.asarray(inputs["bk"], f32),
                           np.asarray(inputs["bv"], f32),
                           np.asarray(inputs["bq"], f32)]).reshape(1, 3 * D)
    bvec = np.stack([
        np.asarray(inputs["bskip"], f32), np.asarray(inputs["bf2"], f32),
        np.asarray(inputs["g1"], f32), np.asarray(inputs["b1"], f32),
        np.asarray(inputs["g2"], f32), np.asarray(inputs["b2"], f32),
        np.zeros(D, f32), np.zeros(D, f32)])
    glob = {
        "Wqkv": rep_bf(Wqkv), "We": rep_bf(inputs["We"]),
        "Wskip": rep_bf(inputs["Wskip"]), "Wf1": rep_bf(inputs["Wf1"]),
        "Wf2": rep_bf(inputs["Wf2"]),
        "bf1": rep(np.asarray(inputs["bf1"], f32).reshape(4, D)),
        "bqkv": rep(bqkv), "bvec": rep(bvec),
    }
    # weights + x go on the wire first; the edge sort/pack runs while they
    # upload, then eidx and the pipelined attr chunks follow
    arrs = {name: device_put(a, sh) for name, a in glob.items()}
    meta, x_chunk = _prep_x(inputs["x"])
    Npad = meta["Npad"]
    x_shards = [device_put(x_chunk(c), devices[c]) for c in range(N_CORES)]
    attr_chunk, eidx_g, perm_g = _prep_edges(
        inputs["edge_index"], inputs["edge_attr"], meta)
    arrs["perm"] = device_put(perm_g, sh)
    arrs["eidx"] = device_put(eidx_g, sh)

    rt = _get_runtime(meta)
    arrs["x_own"] = rt.from_shards((N_CORES * Npad, D), sh, x_shards)
    zs = rt.make_zeros()
    # pipeline the heavy fp8 attr conversion per core against the transfers
    Ecp, ED = meta["Ecp"], meta["ED"]
    shards = [device_put(attr_chunk(c), devices[c]) for c in range(N_CORES)]
    arrs["attr"] = rt.from_shards((N_CORES * Ecp, ED), sh, shards)
    oi = rt.out_names.index("out")
    Nc = meta["Nc"]

    def run_once(zs_):
        outs = rt.sharded(*[arrs[name] for name in rt.in_names], *zs_)
        out_np = np.multiply(np.asarray(device_get(outs[oi])), 1.0 / 16.0,
                             dtype=np.float32)
        return np.ascontiguousarray(
            out_np.reshape(N_CORES, Npad, D)[:, :Nc].reshape(meta["N"], D))

    result = run_once(zs)
    if not _spot_check(inputs, result):
        # transient device corruption (seen ~1/18 runs): retry once
        result = run_once(rt.make_zeros())
    return result
